# revision 4
# baseline (speedup 1.0000x reference)
"""Distributed TRN2 kernel for nn_CustomFullyConnectedLayerSoftmax.

Math: the reference's scatter-add builds W[r, c] = V_scaled[(r-c) % 2048, c]
(each (r, c) hit exactly once -> pure permutation), then out = x @ W.T.
So out[:, r] needs column r of W.T, i.e. W.T[c, r] = V_scaled[(r-c)%2048, c].

Sharding: output columns r are split across 8 cores (256 each). Core i
receives B_i = W.T[:, 256*i : 256*(i+1)] as a dense [2048, 256] operand,
interleaved with the replicated x.T into a single input tensor laid out in
SBUF geometry: IN[p, k, 0:32] = x.T[k*128+p, :], IN[p, k, 32:288] =
B_i[k*128+p, :]. Each core computes its disjoint out[:, 256*i:256*(i+1)] =
x @ B_i with 16 accumulating matmuls -- no collectives; host concatenates
the 8 slices.

Pipeline (v2, all on one HWDGE ring so chunks stream back-to-back at line
rate with no cross-queue packet interleave):
  Sync:   chunked input DMAs (small first chunk -> PE starts early; small
          last chunk -> short exposed tail), per-chunk completion sems.
  Tensor: 16 accumulating matmuls, gated per chunk.
  Scalar: PSUM -> SBUF copy with f32->bf16 cast (halves the output DMA),
          output DMA on its own (empty) qAct ring, completion wait.
Host upcasts the bf16 output slice to f32.
"""

import numpy as np

from concourse import bass, bacc, mybir, tile
from concourse import bass_utils

IN_F = 2048
OUT_F = 2048
TOTAL = 2048
BATCH = 32
N_CORES = 8
R_SH = OUT_F // N_CORES          # 256 output columns per core
K_CH = IN_F // 128               # 16 contraction chunks of 128
W_CH = BATCH + R_SH              # 288 = interleaved xT + B row width
K_TOPK = 1844                    # ceil(int(0.9 * 2048 * 2048) / 2048)

# 'f32' or 'bf16' compute/storage dtype for the matmul operands.
DEVICE_DTYPE = "bf16"
# Device->host output dtype ('f32' or 'bf16'; bf16 halves the out DMA and
# the host upcasts -- costs ~1e-3 extra rel err on top of ~2.4e-3).
OUT_DTYPE = "bf16"
# k-slice counts per pipelined chunk (must sum to K_CH).
CHUNKS = [1, 5, 5, 4, 1]
# Keep the end-of-stream wait for the output DMA's completion semaphore.
SAFE_WAIT = True

TRACE = False          # set True (from test.py) to capture neuron-profile
TRACE_KWARGS = {}
LAST_RESULT = None     # BassKernelResults of the most recent run

_graph_cache = {}


def _mybir_dt(key):
    return mybir.dt.float32 if key == "f32" else mybir.dt.bfloat16


def _np_dt(key):
    return mybir.dt.np(_mybir_dt(key))


def _build_graph(dtype_key):
    dt = _mybir_dt(dtype_key)
    odt = _mybir_dt(OUT_DTYPE)
    nc = bass.Bass("TRN2", target_bir_lowering=False, debug=False,
                   enable_asserts=False)

    in_d = nc.dram_tensor("IN", [128, K_CH, W_CH], dt, kind="ExternalInput")
    out_d = nc.dram_tensor("out", [BATCH, R_SH], odt, kind="ExternalOutput")

    assert sum(CHUNKS) == K_CH
    bounds = [0]
    for c in CHUNKS:
        bounds.append(bounds[-1] + c)

    import contextlib
    with contextlib.ExitStack() as stack:
        # One semaphore per chunk: per-engine FIFO order across chunks does
        # not give a safe cumulative count (a fast engine's chunk-j+1 inc
        # can land before a slow engine's chunk-j inc).
        csems = [stack.enter_context(nc.semaphore(f"cs{j}"))
                 for j in range(len(CHUNKS))]
        osem = stack.enter_context(nc.semaphore("osem"))
        msem = stack.enter_context(nc.semaphore("msem"))
        inb = stack.enter_context(
            nc.sbuf_tensor("inb", [128, K_CH, W_CH], dt))
        acc = stack.enter_context(
            nc.psum_tensor("acc", [BATCH, R_SH], mybir.dt.float32))
        ot = stack.enter_context(
            nc.sbuf_tensor("ot", [BATCH, R_SH], odt))
        block = stack.enter_context(nc.Block())

        @block.sync
        def _(sync):
            for j in range(len(CHUNKS)):
                sync.dma_start(
                    inb[:, bounds[j]:bounds[j + 1], :],
                    in_d[:, bounds[j]:bounds[j + 1], :],
                ).then_inc(csems[j], 16)

        @block.tensor
        def _(tensor):
            for j in range(len(CHUNKS)):
                tensor.wait_ge(csems[j], 16)
                for kk in range(bounds[j], bounds[j + 1]):
                    mm = tensor.matmul(
                        acc[:, :],
                        inb[:, kk, 0:BATCH],
                        inb[:, kk, BATCH:W_CH],
                        start=(kk == 0),
                        stop=(kk == K_CH - 1),
                    )
            mm.then_inc(msem, 1)

        @block.scalar
        def _(scalar):
            # Dummy 1-element activation issued before the wait: pulls the
            # lazy ACT_TABLE_LOAD (~1.3us) off the critical tail and into
            # the DMA phase. Reads uninitialized acc/writes ot[0,0] -- both
            # are rewritten by the real copy after msem.
            scalar.copy(ot[0:1, 0:1], acc[0:1, 0:1])
            scalar.wait_ge(msem, 1)
            scalar.copy(ot[:, :], acc[:, :])
            scalar.dma_start(out_d[:, :], ot[:, :]).then_inc(osem, 16)
            # The host reads `out` right after NEFF completion; the output
            # DMA must be complete before this engine stream ends.
            if SAFE_WAIT:
                scalar.wait_ge(osem, 16)

    return nc


def _get_graph(dtype_key):
    key = (dtype_key, OUT_DTYPE, tuple(CHUNKS), SAFE_WAIT)
    if key not in _graph_cache:
        _graph_cache[key] = _build_graph(dtype_key)
    return _graph_cache[key]


def _host_shards(x, V, alpha, dtype_key):
    np_dt = _np_dt(dtype_key)

    a = alpha.astype(np.float64)
    e = np.exp(a - a.max())
    scale = np.clip(K_TOPK * (e / e.sum()), 0.0, 1.0).astype(np.float32)
    Vs = V * scale[:, None]                        # [2048, 2048] f32

    # W.T[c, r] = Vs[(r - c) % 2048, c]; with Vt = Vs.T duplicated along
    # columns, row c of W.T is the window Vt2[c, 2048-c : 4096-c] -> a
    # shear expressible as a strided view of the flat buffer.
    Vt2 = np.concatenate([Vs.T, Vs.T], axis=1)     # [2048, 4096]
    flat = np.ascontiguousarray(Vt2).reshape(-1)
    WT = np.lib.stride_tricks.as_strided(
        flat[TOTAL:], shape=(IN_F, OUT_F),
        strides=((2 * TOTAL - 1) * 4, 4))

    xT = np.ascontiguousarray(x.T)                 # [2048, 32]
    # [128, K_CH, BATCH]
    xT_dev = xT.reshape(K_CH, 128, BATCH).transpose(1, 0, 2)

    in_maps = []
    for i in range(N_CORES):
        Bi = np.asarray(WT[:, i * R_SH:(i + 1) * R_SH])   # [2048, 256]
        Bi_dev = Bi.reshape(K_CH, 128, R_SH).transpose(1, 0, 2)
        merged = np.empty((128, K_CH, W_CH), dtype=np_dt)
        merged[:, :, :BATCH] = xT_dev
        merged[:, :, BATCH:] = Bi_dev
        in_maps.append({"IN": merged})
    return in_maps


def kernel(x, V, alpha):
    global LAST_RESULT
    x = np.asarray(x, dtype=np.float32)
    V = np.asarray(V, dtype=np.float32)
    alpha = np.asarray(alpha, dtype=np.float32)

    in_maps = _host_shards(x, V, alpha, DEVICE_DTYPE)
    nc = _get_graph(DEVICE_DTYPE)
    res = bass_utils.run_bass_kernel_spmd(
        nc, in_maps, core_ids=list(range(N_CORES)),
        trace=TRACE, trace_kwargs=TRACE_KWARGS)
    LAST_RESULT = res
    out = np.concatenate(
        [np.asarray(r["out"]).astype(np.float32) for r in res.results],
        axis=1)
    return np.ascontiguousarray(out, dtype=np.float32)


# revision 7
# speedup vs baseline: 1.0133x; 1.0133x over previous
"""Distributed TRN2 kernel for nn_CustomFullyConnectedLayerSoftmax.

Math: the reference's scatter-add builds W[r, c] = V_scaled[(r-c) % 2048, c]
(each (r, c) hit exactly once -> pure permutation), then out = x @ W.T.
So out[:, r] needs column r of W.T, i.e. W.T[c, r] = V_scaled[(r-c)%2048, c].

Sharding: output columns r are split across 8 cores (256 each). Core i
receives B_i = W.T[:, 256*i : 256*(i+1)] as a dense [2048, 256] operand,
interleaved with the replicated x.T into a single input tensor laid out in
SBUF geometry: IN[p, k, 0:32] = x.T[k*128+p, :], IN[p, k, 32:288] =
B_i[k*128+p, :]. Each core computes its disjoint out[:, 256*i:256*(i+1)] =
x @ B_i with 16 accumulating matmuls -- no collectives; host concatenates
the 8 slices.

Pipeline (v2, all on one HWDGE ring so chunks stream back-to-back at line
rate with no cross-queue packet interleave):
  Sync:   chunked input DMAs (small first chunk -> PE starts early; small
          last chunk -> short exposed tail), per-chunk completion sems.
  Tensor: 16 accumulating matmuls, gated per chunk.
  Scalar: PSUM -> SBUF copy with f32->bf16 cast (halves the output DMA),
          output DMA on its own (empty) qAct ring, completion wait.
Host upcasts the bf16 output slice to f32.
"""

import numpy as np

from concourse import bass, bacc, mybir, tile
from concourse import bass_utils

IN_F = 2048
OUT_F = 2048
TOTAL = 2048
BATCH = 32
N_CORES = 8
R_SH = OUT_F // N_CORES          # 256 output columns per core
K_CH = IN_F // 128               # 16 contraction chunks of 128
W_CH = BATCH + R_SH              # 288 = interleaved xT + B row width
K_TOPK = 1844                    # ceil(int(0.9 * 2048 * 2048) / 2048)

# 'f32' or 'bf16' compute/storage dtype for the matmul operands.
DEVICE_DTYPE = "bf16"
# Device->host output dtype ('f32' or 'bf16'; bf16 halves the out DMA and
# the host upcasts -- costs ~1e-3 extra rel err on top of ~2.4e-3).
OUT_DTYPE = "bf16"
# k-slice counts per pipelined chunk (must sum to K_CH).
CHUNKS = [1, 5, 5, 4, 1]
# Keep the end-of-stream wait for the output DMA's completion semaphore.
SAFE_WAIT = True
# Emit engine streams inside a bass Block (entry branches + exit barrier)
# or directly into the main basic block (straight-line, no barrier).
USE_BLOCK = False

TRACE = False          # set True (from test.py) to capture neuron-profile
TRACE_KWARGS = {}
LAST_RESULT = None     # BassKernelResults of the most recent run

_graph_cache = {}


def _mybir_dt(key):
    return mybir.dt.float32 if key == "f32" else mybir.dt.bfloat16


def _np_dt(key):
    return mybir.dt.np(_mybir_dt(key))


def _build_graph(dtype_key):
    dt = _mybir_dt(dtype_key)
    odt = _mybir_dt(OUT_DTYPE)
    nc = bass.Bass("TRN2", target_bir_lowering=False, debug=False,
                   enable_asserts=False)

    in_d = nc.dram_tensor("IN", [128, K_CH, W_CH], dt, kind="ExternalInput")
    out_d = nc.dram_tensor("out", [BATCH, R_SH], odt, kind="ExternalOutput")

    assert sum(CHUNKS) == K_CH
    bounds = [0]
    for c in CHUNKS:
        bounds.append(bounds[-1] + c)

    import contextlib
    with contextlib.ExitStack() as stack:
        # One semaphore per chunk: per-engine FIFO order across chunks does
        # not give a safe cumulative count (a fast engine's chunk-j+1 inc
        # can land before a slow engine's chunk-j inc).
        csems = [stack.enter_context(nc.semaphore(f"cs{j}"))
                 for j in range(len(CHUNKS))]
        osem = stack.enter_context(nc.semaphore("osem"))
        msem = stack.enter_context(nc.semaphore("msem"))
        inb = stack.enter_context(
            nc.sbuf_tensor("inb", [128, K_CH, W_CH], dt))
        acc = stack.enter_context(
            nc.psum_tensor("acc", [BATCH, R_SH], mybir.dt.float32))
        ot = stack.enter_context(
            nc.sbuf_tensor("ot", [BATCH, R_SH], odt))
        if USE_BLOCK:
            block = stack.enter_context(nc.Block())

        def body_sync(sync):
            for j in range(len(CHUNKS)):
                sync.dma_start(
                    inb[:, bounds[j]:bounds[j + 1], :],
                    in_d[:, bounds[j]:bounds[j + 1], :],
                ).then_inc(csems[j], 16)

        def body_tensor(tensor):
            mm = None
            for j in range(len(CHUNKS)):
                tensor.wait_ge(csems[j], 16)
                for kk in range(bounds[j], bounds[j + 1]):
                    mm = tensor.matmul(
                        acc[:, :],
                        inb[:, kk, 0:BATCH],
                        inb[:, kk, BATCH:W_CH],
                        start=(kk == 0),
                        stop=(kk == K_CH - 1),
                    )
            mm.then_inc(msem, 1)

        def body_scalar(scalar):
            # Dummy 1-element activation issued before the wait: pulls the
            # lazy ACT_TABLE_LOAD (~1.3us) off the critical tail and into
            # the DMA phase. Reads uninitialized acc/writes ot[0,0] -- both
            # are rewritten by the real copy after msem.
            scalar.copy(ot[0:1, 0:1], acc[0:1, 0:1])
            scalar.wait_ge(msem, 1)
            scalar.copy(ot[:, :], acc[:, :])
            scalar.dma_start(out_d[:, :], ot[:, :]).then_inc(osem, 16)
            # The host reads `out` right after NEFF completion; the output
            # DMA must be complete before this engine stream ends.
            if SAFE_WAIT:
                scalar.wait_ge(osem, 16)

        if USE_BLOCK:
            block.sync(body_sync)
            block.tensor(body_tensor)
            block.scalar(body_scalar)
        else:
            body_sync(nc.sync)
            body_tensor(nc.tensor)
            body_scalar(nc.scalar)

    return nc


def _get_graph(dtype_key):
    key = (dtype_key, OUT_DTYPE, tuple(CHUNKS), SAFE_WAIT, USE_BLOCK)
    if key not in _graph_cache:
        _graph_cache[key] = _build_graph(dtype_key)
    return _graph_cache[key]


def _host_shards(x, V, alpha, dtype_key):
    np_dt = _np_dt(dtype_key)

    a = alpha.astype(np.float64)
    e = np.exp(a - a.max())
    scale = np.clip(K_TOPK * (e / e.sum()), 0.0, 1.0).astype(np.float32)
    Vs = V * scale[:, None]                        # [2048, 2048] f32

    # W.T[c, r] = Vs[(r - c) % 2048, c]; with Vt = Vs.T duplicated along
    # columns, row c of W.T is the window Vt2[c, 2048-c : 4096-c] -> a
    # shear expressible as a strided view of the flat buffer.
    Vt2 = np.concatenate([Vs.T, Vs.T], axis=1)     # [2048, 4096]
    flat = np.ascontiguousarray(Vt2).reshape(-1)
    WT = np.lib.stride_tricks.as_strided(
        flat[TOTAL:], shape=(IN_F, OUT_F),
        strides=((2 * TOTAL - 1) * 4, 4))

    xT = np.ascontiguousarray(x.T)                 # [2048, 32]
    # [128, K_CH, BATCH]
    xT_dev = xT.reshape(K_CH, 128, BATCH).transpose(1, 0, 2)

    in_maps = []
    for i in range(N_CORES):
        Bi = np.asarray(WT[:, i * R_SH:(i + 1) * R_SH])   # [2048, 256]
        Bi_dev = Bi.reshape(K_CH, 128, R_SH).transpose(1, 0, 2)
        merged = np.empty((128, K_CH, W_CH), dtype=np_dt)
        merged[:, :, :BATCH] = xT_dev
        merged[:, :, BATCH:] = Bi_dev
        in_maps.append({"IN": merged})
    return in_maps


def kernel(x, V, alpha):
    global LAST_RESULT
    x = np.asarray(x, dtype=np.float32)
    V = np.asarray(V, dtype=np.float32)
    alpha = np.asarray(alpha, dtype=np.float32)

    in_maps = _host_shards(x, V, alpha, DEVICE_DTYPE)
    nc = _get_graph(DEVICE_DTYPE)
    res = bass_utils.run_bass_kernel_spmd(
        nc, in_maps, core_ids=list(range(N_CORES)),
        trace=TRACE, trace_kwargs=TRACE_KWARGS)
    LAST_RESULT = res
    out = np.concatenate(
        [np.asarray(r["out"]).astype(np.float32) for r in res.results],
        axis=1)
    return np.ascontiguousarray(out, dtype=np.float32)


# revision 15
# speedup vs baseline: 1.1084x; 1.0939x over previous
"""Distributed TRN2 kernel for nn_CustomFullyConnectedLayerSoftmax.

Math: the reference's scatter-add builds W[r, c] = V_scaled[(r-c) % 2048, c]
(each (r, c) hit exactly once -> pure permutation), then out = x @ W.T.
So out[:, r] needs column r of W.T, i.e. W.T[c, r] = V_scaled[(r-c)%2048, c].

Sharding: output columns r are split across 8 cores (256 each). Core i
receives B_i = W.T[:, 256*i : 256*(i+1)] as a dense [2048, 256] operand,
interleaved with the replicated x.T into a single input tensor laid out in
SBUF geometry: IN[p, k, 0:32] = x.T[k*128+p, :], IN[p, k, 32:288] =
B_i[k*128+p, :]. Each core computes its disjoint out[:, 256*i:256*(i+1)] =
x @ B_i with 16 accumulating matmuls -- no collectives; host concatenates
the 8 slices.

Pipeline (final config, measured via within-batch A/B sweeps -- chip-wide
DVFS drift of +-20% across minutes makes cross-invocation timing
comparisons meaningless):
  Sync:   4 chunked input DMAs on one HWDGE ring (back-to-back descriptor
          streams, no cross-queue packet interleave), per-chunk sems.
  Tensor: 16 accumulating matmuls, gated per chunk (~213ns each; the HAM
          throttle never lifts in this environment, warmup is useless).
  Scalar: dummy 1-elem activation at stream start preloads the lazy ACT
          table (~1.3us) off the tail; after msem: PSUM->SBUF copy with
          f32->bf16 cast, output DMA on the idle qAct ring (its issue
          slice overlaps the ACTIVATE via the NX queue), completion wait.
Host upcasts the bf16 output slice to f32 (~1e-3 extra rel err).

No bass Block: engine streams are emitted straight into the main basic
block -- skips the block-entry branches and exit barrier (~0.9us).

SAFE_WAIT must stay True: without the final osem wait the output DMA's 16
sem increments race the runtime epilogue's global semaphore clear and can
leave nonzero values that poison the NEXT execution's waits (observed:
copy/out-DMA firing before the matmuls, silently returning the previous
run's output).
"""

import numpy as np

from concourse import bass, bacc, mybir, tile
from concourse import bass_utils

IN_F = 2048
OUT_F = 2048
TOTAL = 2048
BATCH = 32
N_CORES = 8
R_SH = OUT_F // N_CORES          # 256 output columns per core
K_CH = IN_F // 128               # 16 contraction chunks of 128
W_CH = BATCH + R_SH              # 288 = interleaved xT + B row width
K_TOPK = 1844                    # ceil(int(0.9 * 2048 * 2048) / 2048)

# 'f32' or 'bf16' compute/storage dtype for the matmul operands.
DEVICE_DTYPE = "bf16"
# Device->host output dtype ('f32' or 'bf16'; bf16 halves the out DMA and
# the host upcasts -- costs ~1e-3 extra rel err on top of ~2.4e-3).
OUT_DTYPE = "bf16"
# k-slice counts per pipelined chunk (must sum to K_CH).
CHUNKS = [4, 4, 4, 4]
# Keep the end-of-stream wait for the output DMA's completion semaphore.
SAFE_WAIT = True
# Emit engine streams inside a bass Block (entry branches + exit barrier)
# or directly into the main basic block (straight-line, no barrier).
USE_BLOCK = False
# 'act': Scalar does PSUM->SBUF copy + out DMA (needs dummy act to preload
# the ACT table). 'dve': Vector copies, Sync issues the out DMA (no table).
COPY_ENGINE = "act"

TRACE = False          # set True (from test.py) to capture neuron-profile
TRACE_KWARGS = {}
LAST_RESULT = None     # BassKernelResults of the most recent run

_graph_cache = {}


def _mybir_dt(key):
    return mybir.dt.float32 if key == "f32" else mybir.dt.bfloat16


def _np_dt(key):
    return mybir.dt.np(_mybir_dt(key))


def _build_graph(dtype_key):
    dt = _mybir_dt(dtype_key)
    odt = _mybir_dt(OUT_DTYPE)
    nc = bass.Bass("TRN2", target_bir_lowering=False, debug=False,
                   enable_asserts=False)

    in_d = nc.dram_tensor("IN", [128, K_CH, W_CH], dt, kind="ExternalInput")
    out_d = nc.dram_tensor("out", [BATCH, R_SH], odt, kind="ExternalOutput")

    assert sum(CHUNKS) == K_CH
    bounds = [0]
    for c in CHUNKS:
        bounds.append(bounds[-1] + c)

    import contextlib
    with contextlib.ExitStack() as stack:
        # One semaphore per chunk: per-engine FIFO order across chunks does
        # not give a safe cumulative count (a fast engine's chunk-j+1 inc
        # can land before a slow engine's chunk-j inc).
        csems = [stack.enter_context(nc.semaphore(f"cs{j}"))
                 for j in range(len(CHUNKS))]
        osem = stack.enter_context(nc.semaphore("osem"))
        msem = stack.enter_context(nc.semaphore("msem"))
        psem = (stack.enter_context(nc.semaphore("psem"))
                if COPY_ENGINE == "dve" else None)
        inb = stack.enter_context(
            nc.sbuf_tensor("inb", [128, K_CH, W_CH], dt))
        acc = stack.enter_context(
            nc.psum_tensor("acc", [BATCH, R_SH], mybir.dt.float32))
        ot = stack.enter_context(
            nc.sbuf_tensor("ot", [BATCH, R_SH], odt))
        if USE_BLOCK:
            block = stack.enter_context(nc.Block())

        def body_sync(sync):
            for j in range(len(CHUNKS)):
                sync.dma_start(
                    inb[:, bounds[j]:bounds[j + 1], :],
                    in_d[:, bounds[j]:bounds[j + 1], :],
                ).then_inc(csems[j], 16)
            if COPY_ENGINE == "dve":
                sync.wait_ge(psem, 1)
                sync.dma_start(out_d[:, :], ot[:, :]).then_inc(osem, 16)
                if SAFE_WAIT:
                    sync.wait_ge(osem, 16)

        def body_tensor(tensor):
            mm = None
            for j in range(len(CHUNKS)):
                tensor.wait_ge(csems[j], 16)
                for kk in range(bounds[j], bounds[j + 1]):
                    mm = tensor.matmul(
                        acc[:, :],
                        inb[:, kk, 0:BATCH],
                        inb[:, kk, BATCH:W_CH],
                        start=(kk == 0),
                        stop=(kk == K_CH - 1),
                    )
            mm.then_inc(msem, 1)

        def body_scalar(scalar):
            # Dummy 1-element activation issued before the wait: pulls the
            # lazy ACT_TABLE_LOAD (~1.3us) off the critical tail and into
            # the DMA phase. Reads uninitialized acc/writes ot[0,0] -- both
            # are rewritten by the real copy after msem.
            scalar.copy(ot[0:1, 0:1], acc[0:1, 0:1])
            scalar.wait_ge(msem, 1)
            scalar.copy(ot[:, :], acc[:, :])
            scalar.dma_start(out_d[:, :], ot[:, :]).then_inc(osem, 16)
            # The host reads `out` right after NEFF completion; the output
            # DMA must be complete before this engine stream ends.
            if SAFE_WAIT:
                scalar.wait_ge(osem, 16)

        def body_vector(vector):
            vector.wait_ge(msem, 1)
            vector.tensor_copy(ot[:, :], acc[:, :]).then_inc(psem, 1)

        if USE_BLOCK:
            block.sync(body_sync)
            block.tensor(body_tensor)
            if COPY_ENGINE == "dve":
                block.vector(body_vector)
            else:
                block.scalar(body_scalar)
        else:
            body_sync(nc.sync)
            body_tensor(nc.tensor)
            if COPY_ENGINE == "dve":
                body_vector(nc.vector)
            else:
                body_scalar(nc.scalar)

    return nc


def _get_graph(dtype_key):
    key = (dtype_key, OUT_DTYPE, tuple(CHUNKS), SAFE_WAIT, USE_BLOCK,
           COPY_ENGINE)
    if key not in _graph_cache:
        _graph_cache[key] = _build_graph(dtype_key)
    return _graph_cache[key]


def _host_shards(x, V, alpha, dtype_key):
    np_dt = _np_dt(dtype_key)

    a = alpha.astype(np.float64)
    e = np.exp(a - a.max())
    scale = np.clip(K_TOPK * (e / e.sum()), 0.0, 1.0).astype(np.float32)
    Vs = V * scale[:, None]                        # [2048, 2048] f32

    # W.T[c, r] = Vs[(r - c) % 2048, c]; with Vt = Vs.T duplicated along
    # columns, row c of W.T is the window Vt2[c, 2048-c : 4096-c] -> a
    # shear expressible as a strided view of the flat buffer.
    Vt2 = np.concatenate([Vs.T, Vs.T], axis=1)     # [2048, 4096]
    flat = np.ascontiguousarray(Vt2).reshape(-1)
    WT = np.lib.stride_tricks.as_strided(
        flat[TOTAL:], shape=(IN_F, OUT_F),
        strides=((2 * TOTAL - 1) * 4, 4))

    xT = np.ascontiguousarray(x.T)                 # [2048, 32]
    # [128, K_CH, BATCH]
    xT_dev = xT.reshape(K_CH, 128, BATCH).transpose(1, 0, 2)

    in_maps = []
    for i in range(N_CORES):
        Bi = np.asarray(WT[:, i * R_SH:(i + 1) * R_SH])   # [2048, 256]
        Bi_dev = Bi.reshape(K_CH, 128, R_SH).transpose(1, 0, 2)
        merged = np.empty((128, K_CH, W_CH), dtype=np_dt)
        merged[:, :, :BATCH] = xT_dev
        merged[:, :, BATCH:] = Bi_dev
        in_maps.append({"IN": merged})
    return in_maps


def kernel(x, V, alpha):
    global LAST_RESULT
    x = np.asarray(x, dtype=np.float32)
    V = np.asarray(V, dtype=np.float32)
    alpha = np.asarray(alpha, dtype=np.float32)

    in_maps = _host_shards(x, V, alpha, DEVICE_DTYPE)
    nc = _get_graph(DEVICE_DTYPE)
    res = bass_utils.run_bass_kernel_spmd(
        nc, in_maps, core_ids=list(range(N_CORES)),
        trace=TRACE, trace_kwargs=TRACE_KWARGS)
    LAST_RESULT = res
    out = np.concatenate(
        [np.asarray(r["out"]).astype(np.float32) for r in res.results],
        axis=1)
    return np.ascontiguousarray(out, dtype=np.float32)
